# revision 1
# baseline (speedup 1.0000x reference)
"""Trainium2 Bass kernel for the 4-qubit variational-circuit batch evaluator.

Math: the circuit output is exactly out[b, w] = sum_m C[w, m] * F_m(x_b)
where F_m are the 81 products of per-wire features {1, cos x_v, sin x_v}
(Pauli strings with X vanish on the RX product state).  C depends only on
`weights` and is recovered on the host (f64 lstsq against a tiny numpy
re-implementation of the 16-dim circuit).  Terms are pruned by magnitude
(cumulative |C| drop budget 3e-4 — rel err ~4e-5, well under fp32 tolerance).

Device kernel per core (data parallel, batch sharded 8 ways):
  - x shard [131072, 4] viewed as [128 partitions, 1024 samples, 4 ch]
  - DVE add_range_wrap brings sin/cos arguments into [-pi, pi]
  - ACT Sin evaluates cos (shift pi/2) / sin (shift 0) channels in bulk
  - tensor_scalar / scalar_tensor_tensor build the few surviving
    coefficient-weighted products straight into the interleaved out tile,
    greedily balanced between DVE and GPSIMD (gpsimd: 1-input + plain
    tensor_tensor only — scalar_tensor_tensor there breaks neuronx)
  - 8 chunks, each one contiguous 256 KB DRAM block (fully linear DMAs),
    deep-buffered (bufs=8) so the input DMA stream runs ahead of compute
"""
import math
import sys

import numpy as np

sys.path.insert(0, "/opt/trn_rl_repo")

N_QUBITS = 4
N_LAYERS = 2
CNOT_PAIRS = [(i, j) for i in range(N_QUBITS) for j in range(i + 1, N_QUBITS)]
B_TOTAL = 1048576
N_CORES = 8
S_CORE = B_TOTAL // N_CORES        # 131072 samples per core
P = 128
NPP = S_CORE // P                  # 1024 samples per partition
NCHUNK = 8
NB = NPP // NCHUNK                 # samples per partition per chunk

DROP_BUDGET = 3e-4                 # max cumulative |C| pruned away


# ---------------------------------------------------------------- host math
def _circuit_outputs(x, weights):
    """f64 numpy re-implementation of the reference circuit. [B,4] -> [B,4]."""
    B = x.shape[0]
    state = np.zeros((B,) + (2,) * N_QUBITS, dtype=np.complex128)
    state[(slice(None),) + (0,) * N_QUBITS] = 1.0

    def apply_1q(state, gate, wire):
        s = np.moveaxis(state, wire + 1, -1)
        if gate.ndim == 3:
            s = np.einsum("bij,b...j->b...i", gate, s)
        else:
            s = np.einsum("ij,b...j->b...i", gate, s)
        return np.moveaxis(s, -1, wire + 1)

    for w in range(N_QUBITS):
        th = x[:, w] * 0.5
        c = np.cos(th)
        s = -1j * np.sin(th)
        gate = np.stack([np.stack([c, s], -1), np.stack([s, c], -1)], -2)
        state = apply_1q(state, gate, w)
    for l in range(N_LAYERS):
        for (ctrl, tgt) in CNOT_PAIRS:
            s0 = np.take(state, 0, axis=ctrl + 1)
            s1 = np.take(state, 1, axis=ctrl + 1)
            s1 = np.flip(s1, axis=tgt)
            state = np.stack([s0, s1], axis=ctrl + 1)
        for w in range(N_QUBITS):
            a = weights[l, w] * 0.5
            gate = np.array(
                [[np.cos(a), -np.sin(a)], [np.sin(a), np.cos(a)]],
                dtype=np.complex128,
            )
            state = apply_1q(state, gate, w)
    probs = np.abs(state) ** 2
    outs = []
    for w in range(N_QUBITS):
        p = np.moveaxis(probs, w + 1, 1).reshape(B, 2, -1)
        outs.append(p[:, 0].sum(-1) - p[:, 1].sum(-1))
    return np.stack(outs, -1)


def _features(x):
    """Trig features, kron over wires of [1, cos, sin]. [B,4] -> [B,81]."""
    B = x.shape[0]
    F = np.ones((B, 1))
    for v in range(N_QUBITS):
        g = np.stack([np.ones(B), np.cos(x[:, v]), np.sin(x[:, v])], -1)
        F = (F[:, :, None] * g[:, None, :]).reshape(B, -1)
    return F


def _solve_C(weights):
    """[4, 81] coefficient matrix, exact up to f64 lstsq noise (~1e-13)."""
    rng = np.random.default_rng(1234)
    xs = rng.normal(size=(486, N_QUBITS))
    F = _features(xs)
    Y = _circuit_outputs(xs, weights)
    C, *_ = np.linalg.lstsq(F, Y, rcond=None)
    return C.T


def _select_terms(C):
    """Prune smallest coefficients with cumulative |C| <= DROP_BUDGET.

    Returns per-output lists of (coeff, factors) with factors a tuple of
    (wire, kind) and kind in {"c", "s"}.
    """
    flat = np.abs(C).ravel()
    order = np.argsort(flat)
    cum = np.cumsum(flat[order])
    drop = set(order[cum <= DROP_BUDGET].tolist())
    terms = [[] for _ in range(N_QUBITS)]
    for w in range(N_QUBITS):
        for m in range(81):
            if abs(C[w, m]) == 0.0 or (w * 81 + m) in drop:
                continue
            digits = [(m // 27) % 3, (m // 9) % 3, (m // 3) % 3, m % 3]
            factors = tuple(
                (v, "c" if d == 1 else "s")
                for v, d in enumerate(digits)
                if d != 0
            )
            terms[w].append((float(C[w, m]), factors))
    return terms


def _progression(chans):
    """Smallest arithmetic progression (offset, step, count) covering chans."""
    chans = sorted(set(chans))
    if len(chans) == 1:
        return chans[0], 1, 1
    diffs = [b - a for a, b in zip(chans, chans[1:])]
    step = diffs[0]
    for d in diffs[1:]:
        step = math.gcd(step, d)
    count = (chans[-1] - chans[0]) // step + 1
    return chans[0], step, count


# ------------------------------------------------------------ bass program
class _Balancer:
    """Greedy DVE/GPSIMD placement by modeled busy-ns (DVE 0.96 GHz 1x;
    GPSIMD ~line-rate 1-input, ~2x slower 2-input, 1.2 GHz)."""

    def __init__(self, nc, use_gpsimd, gp_two_in=True):
        self.nc = nc
        self.use_gpsimd = use_gpsimd
        self.gp_two_in = gp_two_in
        self.busy = {"v": 0.0, "g": 0.0}

    def _pick(self, cv, cg):
        if not self.use_gpsimd:
            self.busy["v"] += cv
            return self.nc.vector
        if self.busy["v"] + cv <= self.busy["g"] + cg:
            self.busy["v"] += cv
            return self.nc.vector
        self.busy["g"] += cg
        return self.nc.gpsimd

    def one_in(self, n):          # tensor_scalar / copy
        return self._pick((n + 110) / 0.96, (n + 250) / 1.2)

    def two_in(self, n, is_tt=False):  # tensor_tensor / scalar_tensor_tensor
        allow = self.gp_two_in is True or (self.gp_two_in == "tt" and is_tt)
        if not allow:
            self.busy["v"] += (n + 160) / 0.96
            return self.nc.vector
        return self._pick((n + 160) / 0.96, (2 * n + 250) / 1.2)

    def dve_only(self, n):        # custom ops (add_range_wrap)
        self.busy["v"] += (n + 160) / 0.96
        return self.nc.vector


def _build_program(terms, reps=1, use_gpsimd=True, nchunk=NCHUNK,
                   bufs=8, out_eng="sync", gp_two_in="tt"):
    import concourse.bacc as bacc
    import concourse.tile as tile
    from concourse import mybir

    f32 = mybir.dt.float32
    Sin = mybir.ActivationFunctionType.Sin
    mult = mybir.AluOpType.mult
    add = mybir.AluOpType.add
    PI = float(np.pi)
    HALF_PI = float(np.pi / 2)
    nb = NPP // nchunk

    cos_ch = sorted({v for tl in terms for _, fs in tl for v, k in fs if k == "c"})
    sin_ch = sorted({v for tl in terms for _, fs in tl for v, k in fs if k == "s"})

    nc = bacc.Bacc("TRN2", target_bir_lowering=False, debug=False,
                   num_devices=N_CORES)
    x_d = nc.dram_tensor("x", [S_CORE, N_QUBITS], f32, kind="ExternalInput").ap()
    o_d = nc.dram_tensor("out", [S_CORE, N_QUBITS], f32,
                         kind="ExternalOutput").ap()
    # chunk k <-> contiguous DRAM block k; within a block, partition-major.
    # Fully linear DMAs; in/out use the same sample mapping so the kernel
    # stays elementwise-consistent.
    x2 = x_d.rearrange("(k p n) c -> k p (n c)", k=nchunk, p=P)
    o2 = o_d.rearrange("(k p n) c -> k p (n c)", k=nchunk, p=P)
    bal = _Balancer(nc, use_gpsimd, gp_two_in)

    with tile.TileContext(nc) as tc:
        with tc.tile_pool(name="xp", bufs=bufs) as xp, \
             tc.tile_pool(name="trig", bufs=bufs) as trigp, \
             tc.tile_pool(name="tmp", bufs=2 * bufs) as tmpp, \
             tc.tile_pool(name="op", bufs=bufs) as op:
            for k in range(nchunk * reps):
                k = k % nchunk
                xt = xp.tile([P, 4 * nb], f32)
                nc.sync.dma_start(xt[:], x2[k])
                xr = xt[:].rearrange("p (n c) -> p n c", c=4)

                feat = {}
                for kind, chans, shift in (("c", cos_ch, HALF_PI),
                                           ("s", sin_ch, 0.0)):
                    if not chans:
                        continue
                    off, st, cnt = _progression(chans)
                    wt = tmpp.tile([P, cnt * nb], f32, tag=f"w{kind}")
                    wr = wt[:].rearrange("p (n c) -> p n c", c=cnt)
                    src = xr[:, :, off:off + st * cnt:st] if cnt > 1 \
                        else xr[:, :, off]
                    dst = wr[:, :, :] if cnt > 1 else wt[:]
                    bal.dve_only(cnt * nb).add_range_wrap(
                        dst, src, shift=shift, bound=PI, period=2 * PI)
                    tt = trigp.tile([P, cnt * nb], f32, tag=f"t{kind}")
                    nc.scalar.activation(tt[:], wt[:], Sin)
                    trr = tt[:].rearrange("p (n c) -> p n c", c=cnt)
                    for v in chans:
                        feat[(v, kind)] = trr[:, :, (v - off) // st]

                ot = op.tile([P, 4 * nb], f32)
                orr = ot[:].rearrange("p (n c) -> p n c", c=4)

                for w in range(N_QUBITS):
                    tl = sorted(terms[w], key=lambda t: -len(t[1]))
                    out_ap = orr[:, :, w]
                    if not tl:
                        nc.vector.memset(out_ap, 0.0)
                        continue
                    # put one degree-1 term last so it fuses into the
                    # final accumulate as a scalar_tensor_tensor
                    for i in range(len(tl) - 1, -1, -1):
                        if len(tl[i][1]) == 1:
                            tl.append(tl.pop(i))
                            break

                    def emit_product(coeff, fs, dst):
                        """dst = coeff * prod(features)"""
                        aps = [feat[f] for f in fs]
                        if len(aps) == 1:
                            bal.one_in(nb).tensor_scalar(dst, aps[0], coeff,
                                                         None, mult)
                            return
                        if len(aps) == 2:
                            bal.two_in(nb).scalar_tensor_tensor(
                                dst, aps[0], coeff, aps[1], mult, mult)
                            return
                        t = tmpp.tile([P, nb], f32, tag="pp")
                        bal.two_in(nb).scalar_tensor_tensor(
                            t[:], aps[0], coeff, aps[1], mult, mult)
                        for ap_ in aps[2:-1]:
                            t2 = tmpp.tile([P, nb], f32, tag="pp2")
                            bal.two_in(nb, is_tt=True).tensor_tensor(t2[:], t[:], ap_, mult)
                            t = t2
                        bal.two_in(nb, is_tt=True).tensor_tensor(dst, t[:], aps[-1], mult)

                    if len(tl) == 1:
                        coeff, fs = tl[0]
                        if fs:
                            emit_product(coeff, fs, out_ap)
                        else:
                            nc.vector.memset(out_ap, coeff)
                        continue

                    acc = None
                    const_c = 0.0
                    for coeff, fs in tl[:-1]:
                        if not fs:
                            const_c += coeff
                            continue
                        t = tmpp.tile([P, nb], f32, tag=f"acc{w}")
                        emit_product(coeff, fs, t[:])
                        if acc is None:
                            acc = t
                        else:
                            t2 = tmpp.tile([P, nb], f32, tag=f"acc{w}b")
                            bal.two_in(nb, is_tt=True).tensor_tensor(t2[:], acc[:], t[:],
                                                         add)
                            acc = t2
                    coeff, fs = tl[-1]
                    final_dst = out_ap
                    if const_c != 0.0:
                        final_dst_t = tmpp.tile([P, nb], f32, tag=f"fc{w}")
                        final_dst = final_dst_t[:]
                    if acc is None:
                        emit_product(coeff, fs, final_dst)
                    elif len(fs) == 1:
                        bal.two_in(nb).scalar_tensor_tensor(
                            final_dst, feat[fs[0]], coeff, acc[:], mult, add)
                    else:
                        t = tmpp.tile([P, nb], f32, tag=f"lt{w}")
                        emit_product(coeff, fs, t[:])
                        bal.two_in(nb, is_tt=True).tensor_tensor(final_dst, acc[:], t[:],
                                                     add)
                    if const_c != 0.0:
                        bal.one_in(nb).tensor_scalar(out_ap, final_dst,
                                                     const_c, None, add)

                getattr(nc, out_eng).dma_start(o2[k], ot[:])

    nc.compile()
    from concourse.bass_interp import get_hw_module
    nc.m = get_hw_module(nc.m)
    return nc


_CACHE = {}


def _get_program(weights):
    key = np.asarray(weights, dtype=np.float64).tobytes()
    if key not in _CACHE:
        C = _solve_C(np.asarray(weights, dtype=np.float64))
        terms = _select_terms(C)
        _CACHE[key] = _build_program(terms)
    return _CACHE[key]


def kernel(x, weights):
    from concourse import bass_utils

    x = np.asarray(x, dtype=np.float32)
    weights = np.asarray(weights, dtype=np.float32)
    assert x.shape == (B_TOTAL, N_QUBITS), x.shape

    nc = _get_program(weights)
    in_maps = [
        {"x": np.ascontiguousarray(x[c * S_CORE:(c + 1) * S_CORE])}
        for c in range(N_CORES)
    ]
    res = bass_utils.run_bass_kernel_spmd(nc, in_maps,
                                          core_ids=list(range(N_CORES)))
    out = np.concatenate([res.results[c]["out"] for c in range(N_CORES)],
                         axis=0)
    return out.astype(np.float32, copy=False)

